# revision 4
# baseline (speedup 1.0000x reference)
"""MemoryBank MoE-routing kernel for 8 Trainium2 NeuronCores.

Reference semantics (B=16, S=2048, D=1024, M=512, T=256, K=8):
    x0 = x[:, 0, :]                          # [B, D]
    scores = x0 @ memory_router              # [B, M]
    top_vals, top_idx = top_k(scores, 8)     # [B, K]
    w = softmax(top_vals)                    # [B, K]
    combined = sum_k w[b,k] * memory_tokens[top_idx[b,k]]   # [B, T, D]
    out = x;  out[:, 1:T+1, :] = combined

Sharding: data-parallel over batch (2 batches per core), memory_tokens and
memory_router replicated on every core.  No collectives.

Performance model (from per-engine trace analysis): the kernel is
DMA-engine-throughput bound.  Engine-bytes per core = pass-through 14.7 MB
+ gathers 16.8 MB + router 2.1 MB + combined 2.1 MB = 35.7 MB over 16
SDMA engines at ~27 GB/s/engine line rate => ~82 us bulk + ~7.5 us fixed
startup + ~3 us exit.  HBM->HBM (pass-through) costs 2 HBM-bytes per byte
(716 GB/s HBM total), so pass-through may use at most ~2/3 of engine time
without throttling -- it must time-share with the gathers.

Key scheduling facts (measured):
  * SDMA engines round-robin between queues at PACKET granularity
    (~9-17 descriptors).  Pass-through as one big contiguous transfer
    gets 57 KB descriptors and starves the 8 KB-descriptor gathers 8:1.
    => pass-through is CHUNKED into ~1 MB dma_starts so both queues have
    8 KB descriptors and split engine time ~50/50.
  * A 2-input DVE op (FMA into the accumulator) runs at half rate in
    f32 (2.35 us per [128,2048]); the 16-step chain is 38 us of serial
    DVE.  => gathers cast f32->bf16 during the DMA (SWDGE cast) and the
    chain accumulates in bf16 at 2x, converting to f32 on the last step.
  * Routing latency: scores computed as a [2, M] matmul chain (8
    accumulating matmuls with [128, 2] stationary, ~0.9 us each incl.
    the fp32 2-pass penalty) pipelined behind a 4-quarter router load;
    top-8/softmax/row-id math on 2 partitions; only the [2, 32]
    (idx, w) payload is broadcast to 128 partitions with one ones-matmul
    (the indirect-gather offset AP must be per-partition).
  * Combined writes ride the ACT HWDGE ring: they fire the moment their
    FMA chain completes and never block the SWDGE descriptor stream.
"""

import numpy as np

import concourse.bass as bass
import concourse.bacc as bacc
import concourse.mybir as mybir
from concourse import tile
from concourse.bass_utils import run_bass_kernel_spmd

N_CORES = 8
B, S, D = 16, 2048, 1024
M, T = 512, 256
K = 8
B_LOC = B // N_CORES  # batches per core
KT = D // 128         # contraction tiles for the router matmul
NQ = 4                # router load quarters
PT_ROWS = 256         # pass-through chunk rows (~1 MB => 8 KB descriptors)

F32 = mybir.dt.float32
BF16 = mybir.dt.bfloat16
I32 = mybir.dt.int32
U32 = mybir.dt.uint32


def build_program():
    nc = bacc.Bacc(
        "TRN2",
        target_bir_lowering=False,
        debug=False,
        num_devices=N_CORES,
    )

    x = nc.dram_tensor("x", [B_LOC, S, D], F32, kind="ExternalInput")
    mem = nc.dram_tensor("mem", [M, T, D], F32, kind="ExternalInput")
    router = nc.dram_tensor("router", [D, M], F32, kind="ExternalInput")
    out = nc.dram_tensor("out", [B_LOC, S, D], F32, kind="ExternalOutput")

    KQ = KT // NQ  # kt tiles per router quarter

    with tile.TileContext(nc) as tc:
        with (
            tc.tile_pool(name="sbuf", bufs=1) as sp,
            tc.tile_pool(name="gpool", bufs=16) as gp,
            tc.tile_pool(name="psum", bufs=1, space="PSUM") as pp,
        ):
            # ---- 1. routing inputs on the sync FIFO: x0 (8 KB) first,
            # then the router in quarters so the scores matmuls pipeline
            # behind the loads.  router viewed (p, kt, m) with
            # d = p*KT + kt: partition p holds 16 KiB contiguous DRAM. ----
            x0t = sp.tile([128, B_LOC * KT], F32)
            nc.sync.dma_start(
                out=x0t[:].rearrange("p (b kt) -> p b kt", b=B_LOC),
                in_=x[:, 0, :].rearrange("b (p kt) -> p b kt", kt=KT),
            )
            rview = router[:, :].rearrange("(p kt) m -> p (kt m)", p=128)
            wts = []
            for h in range(NQ):
                wt = sp.tile([128, KQ * M], F32, name=f"wt{h}", tag=f"wt{h}")
                nc.sync.dma_start(
                    out=wt[:], in_=rview[:, h * KQ * M : (h + 1) * KQ * M]
                )
                wts.append(wt)

            # fence: keep the routing loads ahead of the pass-through in
            # the sync HWDGE FIFO
            tc.no_sync_barrier()

            # ---- 2. pass-through stream on the sync ring (HBM->HBM).
            # Issued as odd/even row halves: the strided AP keeps the
            # descriptors at one row (4 KB) so the SDMA packet round-robin
            # gives the 8 KB-descriptor gathers a fair (better) share,
            # instead of the 8:1 starvation a contiguous transfer's 64 KB
            # descriptors cause. ----
            nc.sync.dma_start(out=out[:, 0, :], in_=x[:, 0, :])
            NPAIR = (S - T - 2) // 2  # rows T+1..S-2 as pairs; S-1 separate
            for b in range(B_LOC):
                xv = x[b, T + 1 : S - 1, :].rearrange("(r two) d -> r (two d)", two=2)
                ov = out[b, T + 1 : S - 1, :].rearrange("(r two) d -> r (two d)", two=2)
                nc.sync.dma_start(out=ov[:, 0:D], in_=xv[:, 0:D])
                nc.sync.dma_start(out=ov[:, D : 2 * D], in_=xv[:, D : 2 * D])
                nc.sync.dma_start(out=out[b, S - 1, :], in_=x[b, S - 1, :])

            # ---- 3. constants (iotas, masks, ones row) ----
            iota_i = sp.tile([128, 1], I32)
            nc.gpsimd.iota(iota_i[:], pattern=[[0, 1]], base=0, channel_multiplier=1)
            iotaf = sp.tile([128, 1], F32)
            nc.vector.tensor_copy(out=iotaf[:], in_=iota_i[:])

            iota2_i = sp.tile([B_LOC, 1], I32)
            nc.gpsimd.iota(iota2_i[:], pattern=[[0, 1]], base=0, channel_multiplier=1)
            m1 = sp.tile([B_LOC, 1], F32)  # [0, 1] partition mask
            nc.vector.tensor_copy(out=m1[:], in_=iota2_i[:])
            m0 = sp.tile([B_LOC, 1], F32)  # [1, 0] partition mask
            nc.vector.tensor_scalar(
                out=m0[:], in0=m1[:], scalar1=-1.0, scalar2=1.0,
                op0=mybir.AluOpType.mult, op1=mybir.AluOpType.add,
            )
            ones_i = sp.tile([B_LOC, 128], I32)
            nc.gpsimd.iota(ones_i[:], pattern=[[0, 128]], base=1, channel_multiplier=0)
            ones2 = sp.tile([B_LOC, 128], F32)
            nc.vector.tensor_copy(out=ones2[:], in_=ones_i[:])

            # ---- 4. scores = x0 @ router as a [2, M] matmul chain.
            # Chunk kt contracts d in {p*KT + kt}: lhsT = x0t columns
            # {kt, KT+kt} (stride KT), rhs = wt quarter slice. ----
            x0v = x0t[:].rearrange("p (b kt) -> p kt b", b=B_LOC)
            scores = pp.tile([B_LOC, M], F32, tag="scores")
            for kt in range(KT):
                nc.tensor.matmul(
                    out=scores[:],
                    lhsT=x0v[:, kt : kt + 1, :],
                    rhs=wts[kt // KQ][:, (kt % KQ) * M : (kt % KQ + 1) * M],
                    start=(kt == 0),
                    stop=(kt == KT - 1),
                )

            # ---- 5. top-8 + softmax on 2 partitions.  vals are in
            # [-4, 4] so exp() is computed without max subtraction
            # (identical ratio after normalization). ----
            vals = sp.tile([B_LOC, K], F32)
            nc.vector.max(out=vals[:], in_=scores[:])
            idx = sp.tile([B_LOC, K], U32)
            nc.vector.max_index(out=idx[:], in_max=vals[:], in_values=scores[:])

            ex = sp.tile([B_LOC, K], F32)
            ssum = sp.tile([B_LOC, 1], F32)
            nc.scalar.activation(
                out=ex[:],
                in_=vals[:],
                func=mybir.ActivationFunctionType.Exp,
                bias=0.0,
                scale=1.0,
                accum_out=ssum[:, 0:1],
            )
            rec = sp.tile([B_LOC, 1], F32)
            nc.vector.reciprocal(rec[:], ssum[:])
            w2 = sp.tile([B_LOC, K], F32)
            nc.vector.tensor_scalar(
                out=w2[:], in0=ex[:], scalar1=rec[:, 0:1], scalar2=None,
                op0=mybir.AluOpType.mult,
            )
            idxf = sp.tile([B_LOC, K], F32)
            nc.vector.tensor_copy(out=idxf[:], in_=idx[:])

            # ---- 6. broadcast (idx, w) to 128 partitions with one
            # ones-matmul.  payload[2, 32]: batch b's (idxf, w) lives in
            # cols [b*16, b*16+16), zero elsewhere via partition masks, so
            # summing over the 2 partitions = block-diagonal broadcast. ----
            payload = sp.tile([B_LOC, 4 * K], F32)
            nc.vector.tensor_scalar_mul(payload[:, 0:K], idxf[:], m0[:, 0:1])
            nc.vector.tensor_scalar_mul(payload[:, K : 2 * K], w2[:], m0[:, 0:1])
            nc.vector.tensor_scalar_mul(payload[:, 2 * K : 3 * K], idxf[:], m1[:, 0:1])
            nc.vector.tensor_scalar_mul(payload[:, 3 * K : 4 * K], w2[:], m1[:, 0:1])
            bcast = pp.tile([128, 4 * K], F32, tag="bcast")
            nc.tensor.matmul(
                out=bcast[:], lhsT=ones2[:], rhs=payload[:], start=True, stop=True
            )

            # ---- 7. per-batch row ids for the gather: rid[p, k] =
            # idx[b,k]*(T/2) + p for mem viewed [(m t2), (j d)] ----
            wsb = sp.tile([128, B_LOC * K], F32)
            ridu_all = {}
            for b in range(B_LOC):
                nc.vector.tensor_copy(
                    out=wsb[:, b * K : (b + 1) * K],
                    in_=bcast[:, (2 * b + 1) * K : (2 * b + 2) * K],
                )
                ridf = sp.tile([128, K], F32, name=f"ridf{b}", tag=f"ridf{b}")
                nc.vector.scalar_tensor_tensor(
                    out=ridf[:],
                    in0=bcast[:, 2 * b * K : (2 * b + 1) * K],
                    scalar=float(T // 2),
                    in1=iotaf[:, 0:1].to_broadcast([128, K]),
                    op0=mybir.AluOpType.mult,
                    op1=mybir.AluOpType.add,
                )
                ridu = sp.tile([128, K], U32, name=f"ridu{b}", tag=f"ridu{b}")
                nc.vector.tensor_copy(out=ridu[:], in_=ridf[:])
                ridu_all[b] = ridu

            # fence: pin the routing DVE ops ahead of the FMA chain in
            # DVE's in-order stream
            tc.no_sync_barrier()

            # ---- 8. gathers (pool SWDGE ring), cast f32->bf16 in the
            # DMA datapath; bf16 DVE FMA chains trail each gather and the
            # last step converts to f32. ----
            mem2 = mem[:, :, :].rearrange("m (t2 j) d -> (m t2) (j d)", j=2)
            acc = {
                b: sp.tile([128, 2 * D], BF16, name=f"acc{b}", tag=f"acc{b}")
                for b in range(B_LOC)
            }
            cmbs = {
                b: sp.tile([128, 2 * D], F32, name=f"cmb{b}", tag=f"cmb{b}")
                for b in range(B_LOC)
            }
            for b in range(B_LOC):
                for k in range(K):
                    g = gp.tile([128, 2 * D], BF16, tag="g")
                    nc.gpsimd.indirect_dma_start(
                        out=g[:],
                        out_offset=None,
                        in_=mem2,
                        in_offset=bass.IndirectOffsetOnAxis(
                            ap=ridu_all[b][:, k : k + 1], axis=0
                        ),
                    )
                    wk = wsb[:, b * K + k : b * K + k + 1]
                    if k == 0:
                        nc.vector.tensor_scalar_mul(acc[b][:], g[:], wk)
                    elif k < K - 1:
                        nc.vector.scalar_tensor_tensor(
                            out=acc[b][:],
                            in0=g[:],
                            scalar=wk,
                            in1=acc[b][:],
                            op0=mybir.AluOpType.mult,
                            op1=mybir.AluOpType.add,
                        )
                    else:
                        nc.vector.scalar_tensor_tensor(
                            out=cmbs[b][:],
                            in0=g[:],
                            scalar=wk,
                            in1=acc[b][:],
                            op0=mybir.AluOpType.mult,
                            op1=mybir.AluOpType.add,
                        )

            # ---- 9. combined writes on the ACT HWDGE ring: each fires
            # the moment its FMA chain completes, overlapping the
            # remaining gathers ----
            for b in range(B_LOC):
                nc.scalar.dma_start(
                    out=out[b, 1 : T + 1, :].rearrange("(p j) d -> p j d", j=2),
                    in_=cmbs[b][:].rearrange("p (j d) -> p j d", j=2),
                )

    nc.compile()
    return nc


def kernel(x, memory_tokens, memory_router):
    nc = build_program()
    in_maps = [
        {
            "x": np.ascontiguousarray(x[c * B_LOC : (c + 1) * B_LOC]),
            "mem": memory_tokens,
            "router": memory_router,
        }
        for c in range(N_CORES)
    ]
    res = run_bass_kernel_spmd(nc, in_maps, list(range(N_CORES)))
    return np.concatenate(
        [res.results[c]["out"] for c in range(N_CORES)], axis=0
    )


# revision 9
# speedup vs baseline: 1.8456x; 1.8456x over previous
"""MemoryBank MoE-routing kernel for 8 Trainium2 NeuronCores.

Reference semantics (B=16, S=2048, D=1024, M=512, T=256, K=8):
    x0 = x[:, 0, :]                          # [B, D]
    scores = x0 @ memory_router              # [B, M]
    top_vals, top_idx = top_k(scores, 8)     # [B, K]
    w = softmax(top_vals)                    # [B, K]
    combined = sum_k w[b,k] * memory_tokens[top_idx[b,k]]   # [B, T, D]
    out = x;  out[:, 1:T+1, :] = combined

Sharding: data-parallel over batch (2 batches per core), memory_tokens and
memory_router replicated on every core.  No collectives.

Performance model (from per-engine trace analysis): the kernel is
DMA-engine-throughput bound.  Engine-bytes per core = pass-through 14.7 MB
+ gathers 16.8 MB + router 2.1 MB + combined 2.1 MB = 35.7 MB over 16
SDMA engines at ~27 GB/s/engine line rate => ~82 us bulk + ~7.5 us fixed
startup + ~3 us exit.  HBM->HBM (pass-through) costs 2 HBM-bytes per byte
(716 GB/s HBM total), so pass-through may use at most ~2/3 of engine time
without throttling -- it must time-share with the gathers.

Key scheduling facts (measured):
  * SDMA engines round-robin between queues at PACKET granularity
    (~9-17 descriptors).  Pass-through as one big contiguous transfer
    gets 57 KB descriptors and starves the 8 KB-descriptor gathers 8:1.
    => pass-through is CHUNKED into ~1 MB dma_starts so both queues have
    8 KB descriptors and split engine time ~50/50.
  * A 2-input DVE op (FMA into the accumulator) runs at half rate in
    f32 (2.35 us per [128,2048]); the 16-step chain is 38 us of serial
    DVE.  => gathers cast f32->bf16 during the DMA (SWDGE cast) and the
    chain accumulates in bf16 at 2x, converting to f32 on the last step.
  * Routing latency: scores computed as a [2, M] matmul chain (8
    accumulating matmuls with [128, 2] stationary, ~0.9 us each incl.
    the fp32 2-pass penalty) pipelined behind a 4-quarter router load;
    top-8/softmax/row-id math on 2 partitions; only the [2, 32]
    (idx, w) payload is broadcast to 128 partitions with one ones-matmul
    (the indirect-gather offset AP must be per-partition).
  * Combined writes ride the ACT HWDGE ring: they fire the moment their
    FMA chain completes and never block the SWDGE descriptor stream.
"""

import numpy as np

import concourse.bass as bass
import concourse.bacc as bacc
import concourse.mybir as mybir
from concourse import tile
from concourse.tile_rust import add_dep_helper
from concourse.bass_utils import run_bass_kernel_spmd

N_CORES = 8
B, S, D = 16, 2048, 1024
M, T = 512, 256
K = 8
B_LOC = B // N_CORES  # batches per core
KT = D // 128         # contraction tiles for the router matmul
NQ = 4                # router load quarters
PT_ROWS = 256         # pass-through chunk rows (~1 MB per chunk)
PT_FREE = 5           # ungated PT chunks covering the routing window

F32 = mybir.dt.float32
BF16 = mybir.dt.bfloat16
I32 = mybir.dt.int32
U32 = mybir.dt.uint32


def build_program():
    nc = bacc.Bacc(
        "TRN2",
        target_bir_lowering=False,
        debug=False,
        num_devices=N_CORES,
    )

    x = nc.dram_tensor("x", [B_LOC, S, D], F32, kind="ExternalInput")
    mem = nc.dram_tensor("mem", [M, T, D], F32, kind="ExternalInput")
    router = nc.dram_tensor("router", [D, M], F32, kind="ExternalInput")
    out = nc.dram_tensor("out", [B_LOC, S, D], F32, kind="ExternalOutput")

    KQ = KT // NQ  # kt tiles per router quarter

    with tile.TileContext(nc) as tc:
        with (
            tc.tile_pool(name="sbuf", bufs=1) as sp,
            tc.tile_pool(name="gpool", bufs=16) as gp,
            tc.tile_pool(name="psum", bufs=1, space="PSUM") as pp,
        ):
            # ---- 1. routing inputs on the sync FIFO: x0 (8 KB) first,
            # then the router in quarters so the scores matmuls pipeline
            # behind the loads.  router viewed (p, kt, m) with
            # d = p*KT + kt: partition p holds 16 KiB contiguous DRAM. ----
            x0t = sp.tile([128, B_LOC * KT], F32)
            nc.sync.dma_start(
                out=x0t[:].rearrange("p (b kt) -> p b kt", b=B_LOC),
                in_=x[:, 0, :].rearrange("b (p kt) -> p b kt", kt=KT),
            )
            rview = router[:, :].rearrange("(p kt) m -> p (kt m)", p=128)
            wts = []
            for h in range(NQ):
                wt = sp.tile([128, KQ * M], F32, name=f"wt{h}", tag=f"wt{h}")
                nc.sync.dma_start(
                    out=wt[:], in_=rview[:, h * KQ * M : (h + 1) * KQ * M]
                )
                wts.append(wt)

            # fence: keep the routing loads ahead of the pass-through in
            # the sync HWDGE FIFO
            tc.no_sync_barrier()

            # ---- 2. pass-through stream on the sync ring (HBM->HBM) in
            # ~1 MB chunks (64 KB descriptors -- anything smaller is
            # HWDGE descriptor-generation bound; anything bigger is the
            # same).  A contiguous PT stream starves the 8 KB-descriptor
            # gathers ~8:1 at the SDMA packet round-robin, so only the
            # first PT_FREE chunks (covering the routing-compute window)
            # are emitted now; the rest are emitted after the gathers and
            # sem-gated on gather completions, draining as ~1 MB bursts
            # interleaved with the gathers. ----
            nc.sync.dma_start(out=out[:, 0, :], in_=x[:, 0, :])
            pt_chunks = []
            for b in range(B_LOC):
                r = T + 1
                while r < S:
                    r1 = min(r + PT_ROWS, S)
                    pt_chunks.append((b, r, r1))
                    r = r1
            for b, r, r1 in pt_chunks[:PT_FREE]:
                nc.sync.dma_start(out=out[b, r:r1, :], in_=x[b, r:r1, :])

            # ---- 3. constants (iotas, masks, ones row) ----
            iota_i = sp.tile([128, 1], I32)
            nc.gpsimd.iota(iota_i[:], pattern=[[0, 1]], base=0, channel_multiplier=1)
            iotaf = sp.tile([128, 1], F32)
            nc.vector.tensor_copy(out=iotaf[:], in_=iota_i[:])

            iota2_i = sp.tile([B_LOC, 1], I32)
            nc.gpsimd.iota(iota2_i[:], pattern=[[0, 1]], base=0, channel_multiplier=1)
            m1 = sp.tile([B_LOC, 1], F32)  # [0, 1] partition mask
            nc.vector.tensor_copy(out=m1[:], in_=iota2_i[:])
            m0 = sp.tile([B_LOC, 1], F32)  # [1, 0] partition mask
            nc.vector.tensor_scalar(
                out=m0[:], in0=m1[:], scalar1=-1.0, scalar2=1.0,
                op0=mybir.AluOpType.mult, op1=mybir.AluOpType.add,
            )
            ones_i = sp.tile([B_LOC, 128], I32)
            nc.gpsimd.iota(ones_i[:], pattern=[[0, 128]], base=1, channel_multiplier=0)
            ones2 = sp.tile([B_LOC, 128], F32)
            nc.vector.tensor_copy(out=ones2[:], in_=ones_i[:])

            # ---- 4. scores = x0 @ router as a [2, M] matmul chain.
            # Chunk kt contracts d in {p*KT + kt}: lhsT = x0t columns
            # {kt, KT+kt} (stride KT), rhs = wt quarter slice. ----
            x0v = x0t[:].rearrange("p (b kt) -> p kt b", b=B_LOC)
            scores = pp.tile([B_LOC, M], F32, tag="scores")
            for kt in range(KT):
                nc.tensor.matmul(
                    out=scores[:],
                    lhsT=x0v[:, kt : kt + 1, :],
                    rhs=wts[kt // KQ][:, (kt % KQ) * M : (kt % KQ + 1) * M],
                    start=(kt == 0),
                    stop=(kt == KT - 1),
                )

            # ---- 5. top-8 + softmax on 2 partitions.  vals are in
            # [-4, 4] so exp() is computed without max subtraction
            # (identical ratio after normalization). ----
            vals = sp.tile([B_LOC, K], F32)
            nc.vector.max(out=vals[:], in_=scores[:])
            idx = sp.tile([B_LOC, K], U32)
            nc.vector.max_index(out=idx[:], in_max=vals[:], in_values=scores[:])

            ex = sp.tile([B_LOC, K], F32)
            ssum = sp.tile([B_LOC, 1], F32)
            nc.scalar.activation(
                out=ex[:],
                in_=vals[:],
                func=mybir.ActivationFunctionType.Exp,
                bias=0.0,
                scale=1.0,
                accum_out=ssum[:, 0:1],
            )
            rec = sp.tile([B_LOC, 1], F32)
            nc.vector.reciprocal(rec[:], ssum[:])
            w2 = sp.tile([B_LOC, K], F32)
            nc.vector.tensor_scalar(
                out=w2[:], in0=ex[:], scalar1=rec[:, 0:1], scalar2=None,
                op0=mybir.AluOpType.mult,
            )
            idxf = sp.tile([B_LOC, K], F32)
            nc.vector.tensor_copy(out=idxf[:], in_=idx[:])

            # ---- 6. broadcast (idx, w) to 128 partitions with one
            # ones-matmul.  payload[2, 32]: batch b's (idxf, w) lives in
            # cols [b*16, b*16+16), zero elsewhere via partition masks, so
            # summing over the 2 partitions = block-diagonal broadcast. ----
            payload = sp.tile([B_LOC, 4 * K], F32)
            nc.vector.tensor_scalar_mul(payload[:, 0:K], idxf[:], m0[:, 0:1])
            nc.vector.tensor_scalar_mul(payload[:, K : 2 * K], w2[:], m0[:, 0:1])
            nc.vector.tensor_scalar_mul(payload[:, 2 * K : 3 * K], idxf[:], m1[:, 0:1])
            nc.vector.tensor_scalar_mul(payload[:, 3 * K : 4 * K], w2[:], m1[:, 0:1])
            bcast = pp.tile([128, 4 * K], F32, tag="bcast")
            nc.tensor.matmul(
                out=bcast[:], lhsT=ones2[:], rhs=payload[:], start=True, stop=True
            )

            # ---- 7. per-batch row ids for the gather: rid[p, k] =
            # idx[b,k]*(T/2) + p for mem viewed [(m t2), (j d)] ----
            wsb = sp.tile([128, B_LOC * K], F32)
            ridu_all = {}
            for b in range(B_LOC):
                nc.vector.tensor_copy(
                    out=wsb[:, b * K : (b + 1) * K],
                    in_=bcast[:, (2 * b + 1) * K : (2 * b + 2) * K],
                )
                ridf = sp.tile([128, K], F32, name=f"ridf{b}", tag=f"ridf{b}")
                nc.vector.scalar_tensor_tensor(
                    out=ridf[:],
                    in0=bcast[:, 2 * b * K : (2 * b + 1) * K],
                    scalar=float(T // 2),
                    in1=iotaf[:, 0:1].to_broadcast([128, K]),
                    op0=mybir.AluOpType.mult,
                    op1=mybir.AluOpType.add,
                )
                ridu = sp.tile([128, K], U32, name=f"ridu{b}", tag=f"ridu{b}")
                nc.vector.tensor_copy(out=ridu[:], in_=ridf[:])
                ridu_all[b] = ridu

            # fence: pin the routing DVE ops ahead of the FMA chain in
            # DVE's in-order stream
            tc.no_sync_barrier()

            # ---- 8. gathers (pool SWDGE ring), cast f32->bf16 in the
            # DMA datapath; bf16 DVE FMA chains trail each gather and the
            # last step converts to f32. ----
            mem2 = mem[:, :, :].rearrange("m (t2 j) d -> (m t2) (j d)", j=2)
            acc = {
                b: sp.tile([128, 2 * D], BF16, name=f"acc{b}", tag=f"acc{b}")
                for b in range(B_LOC)
            }
            cmbs = {
                b: sp.tile([128, 2 * D], F32, name=f"cmb{b}", tag=f"cmb{b}")
                for b in range(B_LOC)
            }
            gather_insts = []
            for b in range(B_LOC):
                for k in range(K):
                    g = gp.tile([128, 2 * D], BF16, tag="g")
                    gi = nc.gpsimd.indirect_dma_start(
                        out=g[:],
                        out_offset=None,
                        in_=mem2,
                        in_offset=bass.IndirectOffsetOnAxis(
                            ap=ridu_all[b][:, k : k + 1], axis=0
                        ),
                    )
                    gather_insts.append(gi)
                    wk = wsb[:, b * K + k : b * K + k + 1]
                    if k == 0:
                        nc.vector.tensor_scalar_mul(acc[b][:], g[:], wk)
                    elif k < K - 1:
                        nc.vector.scalar_tensor_tensor(
                            out=acc[b][:],
                            in0=g[:],
                            scalar=wk,
                            in1=acc[b][:],
                            op0=mybir.AluOpType.mult,
                            op1=mybir.AluOpType.add,
                        )
                    else:
                        nc.vector.scalar_tensor_tensor(
                            out=cmbs[b][:],
                            in0=g[:],
                            scalar=wk,
                            in1=acc[b][:],
                            op0=mybir.AluOpType.mult,
                            op1=mybir.AluOpType.add,
                        )

            # ---- 9. gated pass-through chunks: chunk PT_FREE+i waits
            # gather i's completion, so the sync queue gets ~1 MB of PT
            # work injected per finished gather (engines then burst-drain
            # it and return to the gathers). ----
            for i, (b, r, r1) in enumerate(pt_chunks[PT_FREE:]):
                ci = nc.sync.dma_start(out=out[b, r:r1, :], in_=x[b, r:r1, :])
                add_dep_helper(
                    ci.ins,
                    gather_insts[min(i, len(gather_insts) - 1)].ins,
                    sync=True,
                    reason="throttle pass-through behind gather stream",
                )

            # ---- 10. combined writes on the ACT HWDGE ring: each fires
            # the moment its FMA chain completes, overlapping the
            # remaining gathers ----
            for b in range(B_LOC):
                nc.scalar.dma_start(
                    out=out[b, 1 : T + 1, :].rearrange("(p j) d -> p j d", j=2),
                    in_=cmbs[b][:].rearrange("p (j d) -> p j d", j=2),
                )

    nc.compile()
    return nc


def kernel(x, memory_tokens, memory_router):
    nc = build_program()
    in_maps = [
        {
            "x": np.ascontiguousarray(x[c * B_LOC : (c + 1) * B_LOC]),
            "mem": memory_tokens,
            "router": memory_router,
        }
        for c in range(N_CORES)
    ]
    res = run_bass_kernel_spmd(nc, in_maps, list(range(N_CORES)))
    return np.concatenate(
        [res.results[c]["out"] for c in range(N_CORES)], axis=0
    )


# revision 10
# speedup vs baseline: 1.8636x; 1.0098x over previous
"""MemoryBank MoE-routing kernel for 8 Trainium2 NeuronCores.

Reference semantics (B=16, S=2048, D=1024, M=512, T=256, K=8):
    x0 = x[:, 0, :]                          # [B, D]
    scores = x0 @ memory_router              # [B, M]
    top_vals, top_idx = top_k(scores, 8)     # [B, K]
    w = softmax(top_vals)                    # [B, K]
    combined = sum_k w[b,k] * memory_tokens[top_idx[b,k]]   # [B, T, D]
    out = x;  out[:, 1:T+1, :] = combined

Sharding: data-parallel over batch (2 batches per core), memory_tokens and
memory_router replicated on every core.  No collectives.

Performance model (from per-engine trace analysis): the kernel is
DMA-engine-throughput bound.  Engine-bytes per core = pass-through 14.7 MB
+ gathers 16.8 MB + router 2.1 MB + combined 2.1 MB = 35.7 MB over 16
SDMA engines at ~27 GB/s/engine line rate => ~82 us bulk + ~7.5 us fixed
startup + ~3 us exit.  HBM->HBM (pass-through) costs 2 HBM-bytes per byte
(716 GB/s HBM total), so pass-through may use at most ~2/3 of engine time
without throttling -- it must time-share with the gathers.

Key scheduling facts (measured):
  * SDMA engines round-robin between queues at PACKET granularity
    (~9-17 descriptors).  Pass-through as one big contiguous transfer
    gets 57 KB descriptors and starves the 8 KB-descriptor gathers 8:1.
    => pass-through is CHUNKED into ~1 MB dma_starts so both queues have
    8 KB descriptors and split engine time ~50/50.
  * A 2-input DVE op (FMA into the accumulator) runs at half rate in
    f32 (2.35 us per [128,2048]); the 16-step chain is 38 us of serial
    DVE.  => gathers cast f32->bf16 during the DMA (SWDGE cast) and the
    chain accumulates in bf16 at 2x, converting to f32 on the last step.
  * Routing latency: scores computed as a [2, M] matmul chain (8
    accumulating matmuls with [128, 2] stationary, ~0.9 us each incl.
    the fp32 2-pass penalty) pipelined behind a 4-quarter router load;
    top-8/softmax/row-id math on 2 partitions; only the [2, 32]
    (idx, w) payload is broadcast to 128 partitions with one ones-matmul
    (the indirect-gather offset AP must be per-partition).
  * Combined writes ride the ACT HWDGE ring: they fire the moment their
    FMA chain completes and never block the SWDGE descriptor stream.
"""

import numpy as np

import concourse.bass as bass
import concourse.bacc as bacc
import concourse.mybir as mybir
from concourse import tile
from concourse.tile_rust import add_dep_helper
from concourse.bass_utils import run_bass_kernel_spmd

N_CORES = 8
B, S, D = 16, 2048, 1024
M, T = 512, 256
K = 8
B_LOC = B // N_CORES  # batches per core
KT = D // 128         # contraction tiles for the router matmul
NQ = 4                # router load quarters
PT_ROWS = 256         # pass-through chunk rows (~1 MB per chunk)
PT_FREE = 5           # ungated PT chunks covering the routing window

F32 = mybir.dt.float32
BF16 = mybir.dt.bfloat16
I32 = mybir.dt.int32
U32 = mybir.dt.uint32


def build_program():
    nc = bacc.Bacc(
        "TRN2",
        target_bir_lowering=False,
        debug=False,
        num_devices=N_CORES,
    )

    x = nc.dram_tensor("x", [B_LOC, S, D], F32, kind="ExternalInput")
    mem = nc.dram_tensor("mem", [M, T, D], F32, kind="ExternalInput")
    router = nc.dram_tensor("router", [D, M], F32, kind="ExternalInput")
    out = nc.dram_tensor("out", [B_LOC, S, D], F32, kind="ExternalOutput")

    KQ = KT // NQ  # kt tiles per router quarter

    with tile.TileContext(nc) as tc:
        with (
            tc.tile_pool(name="sbuf", bufs=1) as sp,
            tc.tile_pool(name="gpool", bufs=16) as gp,
            tc.tile_pool(name="psum", bufs=1, space="PSUM") as pp,
        ):
            # ---- 1. routing inputs on the sync FIFO: x0 (8 KB) first,
            # then the router in quarters so the scores matmuls pipeline
            # behind the loads.  router viewed (p, kt, m) with
            # d = p*KT + kt: partition p holds 16 KiB contiguous DRAM. ----
            x0t = sp.tile([128, B_LOC * KT], F32)
            nc.sync.dma_start(
                out=x0t[:].rearrange("p (b kt) -> p b kt", b=B_LOC),
                in_=x[:, 0, :].rearrange("b (p kt) -> p b kt", kt=KT),
            )
            rview = router[:, :].rearrange("(p kt) m -> p (kt m)", p=128)
            wts = []
            for h in range(NQ):
                wt = sp.tile([128, KQ * M], F32, name=f"wt{h}", tag=f"wt{h}")
                nc.sync.dma_start(
                    out=wt[:], in_=rview[:, h * KQ * M : (h + 1) * KQ * M]
                )
                wts.append(wt)

            # fence: keep the routing loads ahead of the pass-through in
            # the sync HWDGE FIFO
            tc.no_sync_barrier()

            # ---- 2. pass-through stream on the sync ring (HBM->HBM) in
            # ~1 MB chunks (64 KB descriptors -- anything smaller is
            # HWDGE descriptor-generation bound; anything bigger is the
            # same).  A contiguous PT stream starves the 8 KB-descriptor
            # gathers ~8:1 at the SDMA packet round-robin, so only the
            # first PT_FREE chunks (covering the routing-compute window)
            # are emitted now; the rest are emitted after the gathers and
            # sem-gated on gather completions, draining as ~1 MB bursts
            # interleaved with the gathers. ----
            nc.sync.dma_start(out=out[:, 0, :], in_=x[:, 0, :])
            pt_chunks = []
            for b in range(B_LOC):
                r = T + 1
                while r < S:
                    r1 = min(r + PT_ROWS, S)
                    pt_chunks.append((b, r, r1))
                    r = r1
            for b, r, r1 in pt_chunks[:PT_FREE]:
                nc.sync.dma_start(out=out[b, r:r1, :], in_=x[b, r:r1, :])

            # ---- 3. constants (iotas, masks, ones row) ----
            iota_i = sp.tile([128, 1], I32)
            nc.gpsimd.iota(iota_i[:], pattern=[[0, 1]], base=0, channel_multiplier=1)
            iotaf = sp.tile([128, 1], F32)
            nc.vector.tensor_copy(out=iotaf[:], in_=iota_i[:])

            iota2_i = sp.tile([B_LOC, 1], I32)
            nc.gpsimd.iota(iota2_i[:], pattern=[[0, 1]], base=0, channel_multiplier=1)
            m1 = sp.tile([B_LOC, 1], F32)  # [0, 1] partition mask
            nc.vector.tensor_copy(out=m1[:], in_=iota2_i[:])
            m0 = sp.tile([B_LOC, 1], F32)  # [1, 0] partition mask
            nc.vector.tensor_scalar(
                out=m0[:], in0=m1[:], scalar1=-1.0, scalar2=1.0,
                op0=mybir.AluOpType.mult, op1=mybir.AluOpType.add,
            )
            ones_i = sp.tile([B_LOC, 128], I32)
            nc.gpsimd.iota(ones_i[:], pattern=[[0, 128]], base=1, channel_multiplier=0)
            ones2 = sp.tile([B_LOC, 128], F32)
            nc.vector.tensor_copy(out=ones2[:], in_=ones_i[:])

            # ---- 4. scores = x0 @ router as a [2, M] matmul chain.
            # Chunk kt contracts d in {p*KT + kt}: lhsT = x0t columns
            # {kt, KT+kt} (stride KT), rhs = wt quarter slice. ----
            x0v = x0t[:].rearrange("p (b kt) -> p kt b", b=B_LOC)
            scores = pp.tile([B_LOC, M], F32, tag="scores")
            for kt in range(KT):
                nc.tensor.matmul(
                    out=scores[:],
                    lhsT=x0v[:, kt : kt + 1, :],
                    rhs=wts[kt // KQ][:, (kt % KQ) * M : (kt % KQ + 1) * M],
                    start=(kt == 0),
                    stop=(kt == KT - 1),
                )

            # ---- 5. top-8 + softmax on 2 partitions.  vals are in
            # [-4, 4] so exp() is computed without max subtraction
            # (identical ratio after normalization). ----
            vals = sp.tile([B_LOC, K], F32)
            nc.vector.max(out=vals[:], in_=scores[:])
            idx = sp.tile([B_LOC, K], U32)
            nc.vector.max_index(out=idx[:], in_max=vals[:], in_values=scores[:])

            ex = sp.tile([B_LOC, K], F32)
            ssum = sp.tile([B_LOC, 1], F32)
            nc.scalar.activation(
                out=ex[:],
                in_=vals[:],
                func=mybir.ActivationFunctionType.Exp,
                bias=0.0,
                scale=1.0,
                accum_out=ssum[:, 0:1],
            )
            rec = sp.tile([B_LOC, 1], F32)
            nc.vector.reciprocal(rec[:], ssum[:])
            w2 = sp.tile([B_LOC, K], F32)
            nc.vector.tensor_scalar(
                out=w2[:], in0=ex[:], scalar1=rec[:, 0:1], scalar2=None,
                op0=mybir.AluOpType.mult,
            )
            idxf = sp.tile([B_LOC, K], F32)
            nc.vector.tensor_copy(out=idxf[:], in_=idx[:])

            # ---- 6. broadcast (idx, w) to 128 partitions with one
            # ones-matmul.  payload[2, 32]: batch b's (idxf, w) lives in
            # cols [b*16, b*16+16), zero elsewhere via partition masks, so
            # summing over the 2 partitions = block-diagonal broadcast. ----
            payload = sp.tile([B_LOC, 4 * K], F32)
            nc.vector.tensor_scalar_mul(payload[:, 0:K], idxf[:], m0[:, 0:1])
            nc.vector.tensor_scalar_mul(payload[:, K : 2 * K], w2[:], m0[:, 0:1])
            nc.vector.tensor_scalar_mul(payload[:, 2 * K : 3 * K], idxf[:], m1[:, 0:1])
            nc.vector.tensor_scalar_mul(payload[:, 3 * K : 4 * K], w2[:], m1[:, 0:1])
            bcast = pp.tile([128, 4 * K], F32, tag="bcast")
            nc.tensor.matmul(
                out=bcast[:], lhsT=ones2[:], rhs=payload[:], start=True, stop=True
            )

            # ---- 7. per-batch row ids for the gather: rid[p, k] =
            # idx[b,k]*(T/2) + p for mem viewed [(m t2), (j d)] ----
            wsb = sp.tile([128, B_LOC * K], F32)
            ridu_all = {}
            for b in range(B_LOC):
                nc.vector.tensor_copy(
                    out=wsb[:, b * K : (b + 1) * K],
                    in_=bcast[:, (2 * b + 1) * K : (2 * b + 2) * K],
                )
                ridf = sp.tile([128, K], F32, name=f"ridf{b}", tag=f"ridf{b}")
                nc.vector.scalar_tensor_tensor(
                    out=ridf[:],
                    in0=bcast[:, 2 * b * K : (2 * b + 1) * K],
                    scalar=float(T // 2),
                    in1=iotaf[:, 0:1].to_broadcast([128, K]),
                    op0=mybir.AluOpType.mult,
                    op1=mybir.AluOpType.add,
                )
                ridu = sp.tile([128, K], U32, name=f"ridu{b}", tag=f"ridu{b}")
                nc.vector.tensor_copy(out=ridu[:], in_=ridf[:])
                ridu_all[b] = ridu

            # fence: pin the routing DVE ops ahead of the FMA chain in
            # DVE's in-order stream
            tc.no_sync_barrier()

            # ---- 8. gathers (pool SWDGE ring), cast f32->bf16 in the
            # DMA datapath; bf16 DVE FMA chains trail each gather and the
            # last step converts to f32. ----
            mem2 = mem[:, :, :].rearrange("m (t2 j) d -> (m t2) (j d)", j=2)
            acc = {
                b: sp.tile([128, 2 * D], BF16, name=f"acc{b}", tag=f"acc{b}")
                for b in range(B_LOC)
            }
            cmbs = {
                b: sp.tile([128, 2 * D], F32, name=f"cmb{b}", tag=f"cmb{b}")
                for b in range(B_LOC)
            }
            gather_insts = []
            for b in range(B_LOC):
                for k in range(K):
                    g = gp.tile([128, 2 * D], BF16, tag="g")
                    gi = nc.gpsimd.indirect_dma_start(
                        out=g[:],
                        out_offset=None,
                        in_=mem2,
                        in_offset=bass.IndirectOffsetOnAxis(
                            ap=ridu_all[b][:, k : k + 1], axis=0
                        ),
                    )
                    gather_insts.append(gi)
                    wk = wsb[:, b * K + k : b * K + k + 1]
                    if k == 0:
                        nc.vector.tensor_scalar_mul(acc[b][:], g[:], wk)
                    elif k < K - 1:
                        nc.vector.scalar_tensor_tensor(
                            out=acc[b][:],
                            in0=g[:],
                            scalar=wk,
                            in1=acc[b][:],
                            op0=mybir.AluOpType.mult,
                            op1=mybir.AluOpType.add,
                        )
                    else:
                        nc.vector.scalar_tensor_tensor(
                            out=cmbs[b][:],
                            in0=g[:],
                            scalar=wk,
                            in1=acc[b][:],
                            op0=mybir.AluOpType.mult,
                            op1=mybir.AluOpType.add,
                        )

            # ---- 9. gated pass-through chunks: chunk PT_FREE+i waits
            # gather i's completion, so the sync queue gets ~1 MB of PT
            # work injected per finished gather (engines then burst-drain
            # it and return to the gathers). ----
            for i, (b, r, r1) in enumerate(pt_chunks[PT_FREE:]):
                ci = nc.sync.dma_start(out=out[b, r:r1, :], in_=x[b, r:r1, :])
                add_dep_helper(
                    ci.ins,
                    gather_insts[min(max(i - 1, 0), len(gather_insts) - 1)].ins,
                    sync=True,
                    reason="throttle pass-through behind gather stream",
                )

            # ---- 10. combined writes on the ACT HWDGE ring: each fires
            # the moment its FMA chain completes, overlapping the
            # remaining gathers ----
            for b in range(B_LOC):
                nc.scalar.dma_start(
                    out=out[b, 1 : T + 1, :].rearrange("(p j) d -> p j d", j=2),
                    in_=cmbs[b][:].rearrange("p (j d) -> p j d", j=2),
                )

    nc.compile()
    return nc


def kernel(x, memory_tokens, memory_router):
    nc = build_program()
    in_maps = [
        {
            "x": np.ascontiguousarray(x[c * B_LOC : (c + 1) * B_LOC]),
            "mem": memory_tokens,
            "router": memory_router,
        }
        for c in range(N_CORES)
    ]
    res = run_bass_kernel_spmd(nc, in_maps, list(range(N_CORES)))
    return np.concatenate(
        [res.results[c]["out"] for c in range(N_CORES)], axis=0
    )
